# revision 15
# baseline (speedup 1.0000x reference)
"""Causal self-attention (B=4, T=2048, HID=2048, NH=16, HD=128) on 8 TRN2 cores.

Sharding: tensor-parallel over heads. Each core owns 2 heads:
  - Wq/Wk/Wv column shards [2048, 256], Wo row shard [256, 2048]
  - computes q,k,v + RoPE + QK-RMSNorm + causal attention for its heads
  - computes its partial c_proj output y_c [8192, 2048]
Host sums the 8 partials (the "all-reduce") and reshapes.

Layout strategy per core:
  - Host uploads xT [2048, 8192] (hidden-major, fp16) so QKV matmuls need
    no on-chip transpose of x: psum[t,n] += xT_tile[k,t].T @ W[k,n].
  - RoPE + RMSNorm run in [t, d] layout (free-dim reductions), q and k
    processed together in one [128, 512] tile.
  - q,k are then PE-transposed to qT/kT [d, t], fp16, SBUF-resident per
    batch (no DRAM scratch round-trip).
  - Scores are computed TRANSPOSED: St[s, tq] = kT_blk.T @ qT so that
    softmax needs no P transpose for PV:  outT[d, tq] += v_blk.T @ P.
  - Softmax skips the max-subtraction: RMSNorm bounds |q.k| <= sqrt(128)
    (the 1/sqrt(d) scale is folded into q's rstd); exp takes a constant
    -1.25 bias so P = exp(s - 1.25) <= 2.4e4 stays in fp16 range
    (softmax-invariant).
  - Denominator via ones-vector matmul (column sums of P), applied to
    outT at PSUM evacuation through a PE outer-product broadcast.
  - Causal mask: multiplicative 0/1 fp16 masks on the 4 diagonal block
    offsets, applied post-exp.
Matmul operands are fp16 (full PE rate + fast weight load); PSUM
accumulation, softmax denominators and all normalization math are fp32.
"""

import sys

if "/opt/trn_rl_repo" not in sys.path:
    sys.path.insert(0, "/opt/trn_rl_repo")

from contextlib import ExitStack

import numpy as np

import concourse.bass as bass
import concourse.tile as tile
from concourse import bacc, mybir
from concourse.bass_utils import run_bass_kernel_spmd

F32 = mybir.dt.float32
F32R = mybir.dt.float32r
F16 = mybir.dt.float16
AF = mybir.ActivationFunctionType
ALU = mybir.AluOpType
AX = mybir.AxisListType

B, T, HID = 4, 2048, 2048
NH, HD = 16, 128
N_CORES = 8
NHC = NH // N_CORES          # heads per core = 2
NC = NHC * HD                # per-core head cols = 256
TM = B * T                   # 8192 tokens
TBB = T // 128               # 16 t-blocks per batch
KC = HID // 128              # 16 contraction chunks
EPS = 1e-5
ROPE_BASE = 10000.0
EXP_BIAS = -1.25             # exp(s + EXP_BIAS): keeps P < 2.4e4 (fp16-safe)


def build_program():
    nc = bacc.Bacc("TRN2", target_bir_lowering=False, debug=False,
                   num_devices=N_CORES)

    xT = nc.dram_tensor("xT", [HID, TM], F16, kind="ExternalInput").ap()
    wqkc = nc.dram_tensor("wqkcat", [HID, 2 * NC], F16, kind="ExternalInput").ap()
    wv = nc.dram_tensor("wv", [HID, NC], F16, kind="ExternalInput").ap()
    wo = nc.dram_tensor("wo", [NC, HID], F16, kind="ExternalInput").ap()
    cosd = nc.dram_tensor("cos", [T, HD // 2], F32, kind="ExternalInput").ap()
    sind = nc.dram_tensor("sin", [T, HD // 2], F32, kind="ExternalInput").ap()
    wqkd = nc.dram_tensor("wqk", [128, NC], F32, kind="ExternalInput").ap()
    maskd = nc.dram_tensor("masks", [4, 128, 512], F16, kind="ExternalInput").ap()
    onescd = nc.dram_tensor("onesc", [128, 1], F16, kind="ExternalInput").ap()
    identd = nc.dram_tensor("ident", [128, 128], F16, kind="ExternalInput").ap()
    onesrd = nc.dram_tensor("onesr", [1, 128], F32R, kind="ExternalInput").ap()
    y = nc.dram_tensor("y", [HID, TM], F32, kind="ExternalOutput").ap()

    with tile.TileContext(nc) as tc, ExitStack() as ctx:
        consts = ctx.enter_context(tc.tile_pool(name="consts", bufs=1))

        # ---- resident constants -------------------------------------------
        wqkc_sb = consts.tile([128, KC, 2 * NC], F16, tag="wqkc")
        wv_sb = consts.tile([128, KC, NC], F16, tag="wv")
        for w_sb, w_d in ((wqkc_sb, wqkc), (wv_sb, wv)):
            nc.sync.dma_start(
                out=w_sb, in_=w_d.rearrange("(k1 k2) n -> k2 k1 n", k2=128))
        wo_sb = consts.tile([128, NHC, HID], F16, tag="wo")
        nc.sync.dma_start(
            out=wo_sb, in_=wo.rearrange("(n1 n2) c -> n2 n1 c", n2=128))
        cos_sb = consts.tile([128, TBB, HD // 2], F32, tag="cos")
        sin_sb = consts.tile([128, TBB, HD // 2], F32, tag="sin")
        nc.sync.dma_start(out=cos_sb,
                          in_=cosd.rearrange("(t1 t2) j -> t2 t1 j", t2=128))
        nc.sync.dma_start(out=sin_sb,
                          in_=sind.rearrange("(t1 t2) j -> t2 t1 j", t2=128))
        wqk_sb = consts.tile([128, NC], F32, tag="wqk")
        nc.sync.dma_start(out=wqk_sb, in_=wqkd)
        mask_sb = consts.tile([128, 4, 512], F16, tag="mask")
        nc.sync.dma_start(out=mask_sb, in_=maskd.rearrange("m p t -> p m t"))
        ident = consts.tile([128, 128], F16, tag="ident")
        nc.sync.dma_start(out=ident, in_=identd)
        ones_col = consts.tile([128, 1], F16, tag="onesc")
        nc.sync.dma_start(out=ones_col, in_=onescd)
        ones_row = consts.tile([1, 128], F32R, tag="onesr")
        nc.sync.dma_start(out=ones_row, in_=onesrd)
        eps_q = consts.tile([128, 1], F32, tag="epsq")
        nc.vector.memset(eps_q, float(HD * EPS))
        eps_k = consts.tile([128, 1], F32, tag="epsk")
        nc.vector.memset(eps_k, float(EPS))
        negc = consts.tile([128, 1], F32, tag="negc")
        nc.vector.memset(negc, EXP_BIAS)

        # ---- PSUM pools ----------------------------------------------------
        ps_qkv = ctx.enter_context(tc.tile_pool(name="ps_qkv", bufs=2, space="PSUM"))
        ps_tr = ctx.enter_context(tc.tile_pool(name="ps_tr", bufs=1, space="PSUM"))
        ps_st = ctx.enter_context(tc.tile_pool(name="ps_st", bufs=2, space="PSUM"))
        ps_acc = ctx.enter_context(tc.tile_pool(name="ps_acc", bufs=3, space="PSUM"))

        # ---- SBUF pools ----------------------------------------------------
        # per-batch resident q/k/v/attention-out (fp16)
        res_pool = ctx.enter_context(tc.tile_pool(name="res", bufs=1))
        xt_pool = ctx.enter_context(tc.tile_pool(name="xt", bufs=2))
        rn_pool = ctx.enter_context(tc.tile_pool(name="rn", bufs=2))
        p_pool = ctx.enter_context(tc.tile_pool(name="pp", bufs=6))
        rec_pool = ctx.enter_context(tc.tile_pool(name="rec", bufs=2))
        y_pool = ctx.enter_context(tc.tile_pool(name="yo", bufs=2))

        def emit_transposes(nrm, tbl, qT_t, kT_t):
            """PE transpose each [128,128] piece -> fp16 resident tiles."""
            for h in range(NHC):
                for src_off, dst in ((h * HD, qT_t[h]), (NC + h * HD, kT_t[h])):
                    t_ps = ps_tr.tile([128, 128], F16, tag="tr")
                    nc.tensor.transpose(t_ps, nrm[:, bass.ds(src_off, HD)],
                                        ident)
                    nc.scalar.copy(dst[:, bass.ts(tbl, 128)], t_ps)

        def qkv_group(b, g, qT_t, kT_t, v_t, pending):
            """QKV + RoPE + RMSNorm for 4 t-blocks (one xt load).

            Transposes are emitted one t-block late so the PE never waits
            on the DVE rope/norm chain."""
            xt = xt_pool.tile([128, KC, 512], F16, tag="xt")
            nc.sync.dma_start(
                out=xt,
                in_=xT[:, bass.ds((b * TBB + 4 * g) * 128, 512)]
                .rearrange("(k1 k2) t -> k2 k1 t", k2=128))
            for sub in range(4):
                tbl = 4 * g + sub
                qk_ps = ps_qkv.tile([128, 2 * NC], F32, tag="ps")
                v_ps = ps_qkv.tile([128, NC], F32, tag="ps")
                for k1 in range(KC):
                    lhs = xt[:, k1, bass.ts(sub, 128)]
                    st, sp = (k1 == 0), (k1 == KC - 1)
                    nc.tensor.matmul(qk_ps, lhs, wqkc_sb[:, k1, :], start=st, stop=sp)
                    nc.tensor.matmul(v_ps, lhs, wv_sb[:, k1, :], start=st, stop=sp)
                if pending[0] is not None:
                    emit_transposes(*pending[0], qT_t, kT_t)
                    pending[0] = None

                # v straight to resident tile (fp16 cast)
                for h in range(NHC):
                    nc.scalar.copy(v_t[h][:, tbl, :], v_ps[:, bass.ts(h, HD)])

                # combined q||k tile [128, 512] fp32
                qk = rn_pool.tile([128, 512], F32, tag="qk")
                nc.scalar.copy(qk, qk_ps)

                # RoPE on 4 groups (q_h0, q_h1, k_h0, k_h1) at once
                v4 = qk.rearrange("p (g half j) -> p g half j", g=4, half=2)
                x1, x2 = v4[:, :, 0, :], v4[:, :, 1, :]
                ct = cos_sb[:, tbl, None, :].broadcast_to([128, 4, HD // 2])
                sn = sin_sb[:, tbl, None, :].broadcast_to([128, 4, HD // 2])
                rot = rn_pool.tile([128, 4, 2, HD // 2], F32, tag="rot")
                tmp = rn_pool.tile([128, 4, HD // 2], F32, tag="tmp")
                r1, r2 = rot[:, :, 0, :], rot[:, :, 1, :]
                nc.vector.tensor_mul(r1, x1, ct)
                nc.vector.tensor_mul(tmp, x2, sn)
                nc.vector.tensor_sub(r1, r1, tmp)
                nc.vector.tensor_mul(r2, x2, ct)
                nc.vector.tensor_mul(tmp, x1, sn)
                nc.vector.tensor_add(r2, r2, tmp)

                # RMSNorm over d per (tensor, head); q gets 1/sqrt(HD) folded
                rv = rot.rearrange("p g half j -> p (g half j)")
                sq = rn_pool.tile([128, 512], F32, tag="sq")
                nc.vector.tensor_mul(sq, rv, rv)
                ssum = rn_pool.tile([128, 4], F32, tag="ssum")
                nc.vector.tensor_reduce(
                    ssum, sq.rearrange("p (g d) -> p g d", g=4),
                    axis=AX.X, op=ALU.add)
                std = rn_pool.tile([128, 4], F32, tag="std")
                nc.scalar.activation(std[:, 0:2], ssum[:, 0:2], AF.Sqrt,
                                     bias=eps_q, scale=1.0)
                nc.scalar.activation(std[:, 2:4], ssum[:, 2:4], AF.Sqrt,
                                     bias=eps_k, scale=1.0 / HD)

                # normalize on the (idle) GPSIMD engine: out = in / std[row]
                nrm = rn_pool.tile([128, 512], F16, tag="nrm")
                for gg in range(4):
                    nc.gpsimd.normalize_recip(
                        nrm[:, bass.ts(gg, HD)],
                        rot[:, gg, :, :].rearrange("p half j -> p (half j)"),
                        std[:, gg:gg + 1])
                # k additionally multiplied by wq*wk (in-place, fp16)
                nc.vector.tensor_mul(nrm[:, NC:2 * NC], nrm[:, NC:2 * NC],
                                     wqk_sb)

                pending[0] = (nrm, tbl)

        def attn_tail(h, j, outT, den, aT_t):
            # 1/den on the small [1,512] vector, broadcast on idle GPSIMD
            rec_sb = rec_pool.tile([1, 512], F32, tag="rec")
            with nc.allow_low_precision(reason="recip of fp32 psum, fp32 out"):
                nc.vector.reciprocal(rec_sb, den)
            bc_sb = rec_pool.tile([128, 512], F32, tag="bcsb")
            nc.gpsimd.partition_broadcast(bc_sb, rec_sb)
            nc.vector.tensor_mul(aT_t[h][:, bass.ts(j, 512)], outT, bc_sb)

        def attn(b, h, qT_t, kT_t, v_t, aT_t, tail):
            for j in range(T // 512):
                outT = ps_acc.tile([128, 512], F32, tag="acc")
                den = ps_acc.tile([1, 512], F32, tag="acc")
                nk = 4 * j + 4
                for k in range(nk):
                    st_ps = ps_st.tile([128, 512], F32, tag="st")
                    nc.tensor.matmul(st_ps, kT_t[h][:, bass.ts(k, 128)],
                                     qT_t[h][:, bass.ts(j, 512)],
                                     start=True, stop=True)
                    p = p_pool.tile([128, 512], F16, tag="p")
                    nc.scalar.activation(p, st_ps, AF.Exp, bias=negc)
                    if k >= 4 * j:
                        nc.vector.tensor_mul(p, p, mask_sb[:, k - 4 * j, :])
                    stt, spp = (k == 0), (k == nk - 1)
                    nc.tensor.matmul(den, ones_col, p, start=stt, stop=spp)
                    nc.tensor.matmul(outT, v_t[h][:, k, :], p, start=stt, stop=spp)
                    if k == 0 and tail[0] is not None:
                        attn_tail(*tail[0], aT_t)
                        tail[0] = None
                tail[0] = (h, j, outT, den)

        def proj_cblock(b, cb, aT_t):
            # yT[c_block, t] = sum_n Wo[n, c_block].T @ aT[n, t]; the Wo
            # chunk stays stationary across two t-supertiles per load.
            y_sb = y_pool.tile([128, T], F32, tag="y")
            for tp in range(2):
                y_ps = [ps_qkv.tile([128, 512], F32, tag="ps",
                                    name=f"yps{b}{cb}{tp}{i}") for i in range(2)]
                for n in range(NHC):
                    for i in range(2):
                        tg = tp * 2 + i
                        nc.tensor.matmul(
                            y_ps[i], wo_sb[:, n, bass.ts(cb, 128)],
                            aT_t[n][:, bass.ts(tg, 512)],
                            start=(n == 0), stop=(n == NHC - 1))
                for i in range(2):
                    nc.any.tensor_copy(
                        y_sb[:, bass.ds((tp * 2 + i) * 512, 512)], y_ps[i])
            nc.sync.dma_start(out=y[bass.ts(cb, 128), bass.ts(b, T)], in_=y_sb)

        for b in range(B):
            qT_t = [res_pool.tile([HD, T], F16, name=f"qT{b}{h}", tag=f"qT{h}")
                    for h in range(NHC)]
            kT_t = [res_pool.tile([HD, T], F16, name=f"kT{b}{h}", tag=f"kT{h}")
                    for h in range(NHC)]
            v_t = [res_pool.tile([128, TBB, HD], F16, name=f"v{b}{h}", tag=f"v{h}")
                   for h in range(NHC)]
            aT_t = [res_pool.tile([HD, T], F16, name=f"aT{b}{h}", tag=f"aT{h}")
                    for h in range(NHC)]
            pending = [None]
            for g in range(TBB // 4):
                qkv_group(b, g, qT_t, kT_t, v_t, pending)
            if pending[0] is not None:
                emit_transposes(*pending[0], qT_t, kT_t)
                pending[0] = None
            tail = [None]
            for h in range(NHC):
                attn(b, h, qT_t, kT_t, v_t, aT_t, tail)
            if tail[0] is not None:
                attn_tail(*tail[0], aT_t)
                tail[0] = None
            for cb in range(HID // 128):
                proj_cblock(b, cb, aT_t)

    nc.compile()
    return nc


_CACHE = {}


def _get_program():
    if "nc" not in _CACHE:
        _CACHE["nc"] = build_program()
    return _CACHE["nc"]


def _host_tables():
    inv = 1.0 / (ROPE_BASE ** (np.arange(0, HD, 2, dtype=np.float32) / HD))
    freqs = np.arange(T, dtype=np.float32)[:, None] * inv[None, :]
    cos = np.cos(freqs).astype(np.float32)
    sin = np.sin(freqs).astype(np.float32)
    m = np.zeros((4, 128, 512), dtype=np.float16)
    s_idx = np.arange(128)[:, None]
    t_idx = np.arange(512)[None, :]
    for off in range(4):
        m[off] = ((off * 128 + s_idx) <= t_idx).astype(np.float16)
    return cos, sin, m


def kernel(x, Wq, Wk, Wv, Wo, q_rms_w, k_rms_w, **_):
    nc = _get_program()
    cos, sin, masks = _host_tables()
    xT = np.ascontiguousarray(
        np.asarray(x, dtype=np.float32).reshape(TM, HID).T).astype(np.float16)
    wqk = (np.asarray(q_rms_w, dtype=np.float32)
           * np.asarray(k_rms_w, dtype=np.float32))
    wqk_b = np.ascontiguousarray(
        np.broadcast_to(np.tile(wqk, NHC)[None, :], (128, NC))).astype(np.float32)

    ones_c = np.ones((128, 1), np.float16)
    ident_h = np.eye(128, dtype=np.float16)
    ones_r = np.ones((1, 128), np.float32)
    in_maps = []
    for c in range(N_CORES):
        cols = slice(c * NC, (c + 1) * NC)
        in_maps.append({
            "xT": xT,
            "wqkcat": np.ascontiguousarray(
                np.concatenate([Wq[:, cols], Wk[:, cols]], axis=1)
            ).astype(np.float16),
            "wv": np.ascontiguousarray(Wv[:, cols]).astype(np.float16),
            "wo": np.ascontiguousarray(Wo[cols, :]).astype(np.float16),
            "cos": cos, "sin": sin, "wqk": wqk_b, "masks": masks,
            "onesc": ones_c, "onesr": ones_r, "ident": ident_h,
        })

    res = run_bass_kernel_spmd(nc, in_maps, list(range(N_CORES)))
    out = res.results[0]["y"].astype(np.float64)
    for c in range(1, N_CORES):
        out += res.results[c]["y"]
    # kernel emits yT [HID, TM]; transpose back on the host
    return np.ascontiguousarray(
        out.astype(np.float32).T).reshape(B, T, HID)
